# revision 33
# baseline (speedup 1.0000x reference)
"""Trainium2 Bass kernel for nn_BrainInspiredRouter.

Math (reference, seq_len==1 attention => attn collapses to the V path):
    attended = x @ (out_proj_w @ Wv).T + (out_proj_w @ bv + out_proj_b)
    h        = relu(attended @ W1[r].T + b1[r])          per route r
    route    = h @ W2[r].T + b2[r]
    gate     = softmax(x @ Wg.T + bg)
    out      = sum_r gate[:, r] * route[:, r, :]

Host-side constant folding (weights only, no activations):
    W1f[r]  = W1[r] @ (out_proj_w @ Wv)      -> h = relu(x @ W1f.T + b1f)
    b1f[r]  = W1[r] @ (out_proj_w@bv + out_proj_b) + b1[r]
    W2cat   = W2.transpose(0,2,1).reshape(R*DH, DOUT)
    out     = (gate*h_flat) @ W2cat + gate @ b2
b2 is folded into GEMM2 as a 33rd uniform k2-tile: hg[32] rows 0..7 hold
the normalized gate, the matching stationary block holds b2 (zero-padded),
so every GEMM2 chain is 33 identical K=128 matmuls (no row-group switch).

Device (per core, batch-sharded 8 ways, 2048 rows each, feature-major "T"
layout so both GEMMs chain without transposes):
    warmup: dummy matmuls on a memset tile keep the PE busy from ~7us so
      the HAM clock-gate reaches 8/8 before real work and never rethrottles.
    gate:  logitsT[8,b] -> E=exp(+bg) -> S bcast to 8 rows via ones8x8 MM
           -> gate_n = E*recip(S) (bf16, pre-normalized; no final 1/S mul)
    gate bcast to 128 partitions: chunk 0 via PE rank-1 MMs (ones[1,128]
      stationary), chunks 1-3 via one DRAM write + one replicating read.
    main loop per 512-col batch chunk:
      GEMM1: psum[h,b] = sum_k w1[k,h-tile] x xT[k,b]
      evict: ACT relu(+b1f) -> f32 tmp; DVE tmp*gate_bcast -> bf16 Hg
      GEMM2: psum[o,b] = sum_k2 w2e[k2,o-tile] x Hg[k2,b]  (33 k2-tiles)
      evict: DVE copy -> f32 -> DMA outT
DMA plan: bulk weights ride the Sync queue in few, large, dep-chained
transfers (x chunk: 1; w1: 4 arrival-ordered waves; w2e: 1/ot); small
latency DMAs (gate round trip, outputs) ride the idle GpSimd queue.
"""

import numpy as np

B, D, DOUT, R = 16384, 1024, 1024, 8
DH = D // 2            # 512
RH = R * DH            # 4096
NCORES = 8
BS = B // NCORES       # 2048 rows per core
CHUNK = 512
NCHUNK = BS // CHUNK   # 4
KT = D // 128          # 8 k-tiles over D
HT = RH // 128         # 32 h-tiles
K2T = RH // 128        # 32 k-tiles over RH (+1 virtual b2 tile)
OT = DOUT // 128       # 8 out-tiles
GRP = DH // 128        # 4 h-tiles per route
RHE = RH + 128         # GEMM2 contraction incl. b2 tile

# w1 arrival waves (in units of h-tiles): small first wave so GEMM1 can
# start early, then big chunks that keep ahead of PE consumption
WAVE_HT = [0, 2, 8, 20, 32]
NWAVE = len(WAVE_HT) - 1
NDUM = 26              # warm-up matmuls (N=256 each, ~213ns cold)
HOLD0 = 2              # chunk-0 h-tiles computed before the gate bcast

_NC_CACHE = {}


def _w1_col(k, ht):
    """column offset of the (k, ht) stationary block in w1_all."""
    for q in range(NWAVE):
        lo, hi = WAVE_HT[q], WAVE_HT[q + 1]
        if lo <= ht < hi:
            off_q = KT * lo * 128
            width = (hi - lo) * 128
            return off_q + k * width + (ht - lo) * 128
    raise AssertionError(ht)


def _build_nc(mm_dt_name="bfloat16"):
    from contextlib import ExitStack

    import concourse.bass as bass
    import concourse.mybir as mybir
    import concourse.tile as tile
    from concourse import bacc

    mm_dt = getattr(mybir.dt, mm_dt_name)
    f32 = mybir.dt.float32
    AF = mybir.ActivationFunctionType

    nc = bacc.Bacc("TRN2", target_bir_lowering=False, debug=False,
                   num_devices=NCORES)

    xT4 = nc.dram_tensor("xT4", [NCHUNK, 128, KT * CHUNK], mm_dt,
                         kind="ExternalInput")
    w1wd = nc.dram_tensor("w1wd", [128, KT * RH], mm_dt,
                          kind="ExternalInput")
    cst = nc.dram_tensor("cst", [128, 1 + HT], f32, kind="ExternalInput")
    w2e = nc.dram_tensor("w2e", [OT, 128, RHE], mm_dt, kind="ExternalInput")
    wgt = nc.dram_tensor("wgt", [128, KT * R], mm_dt, kind="ExternalInput")
    seld = nc.dram_tensor("seld", [R, R * 128], mm_dt, kind="ExternalInput")
    outT = nc.dram_tensor("outT", [OT, 128, BS], f32, kind="ExternalOutput")
    gate_scr = nc.dram_tensor("gate_scr", [NCHUNK, R, CHUNK], mm_dt)

    with tile.TileContext(nc) as tc, ExitStack() as ctx:
        const = ctx.enter_context(tc.tile_pool(name="const", bufs=1))

        # memset consts (no DMA): ready as soon as queues come up
        ones8x8 = const.tile([R, R], mm_dt, tag="ones8x8")
        nc.any.memset(ones8x8[:], 1.0)
        # selector blocks (host-prepared): sel[:, r*128:(r+1)*128] has row r
        # = ones, so sel_r.T @ gate_n replicates gate row r to 128 partitions
        sel = const.tile([R, R * 128], mm_dt, tag="sel")
        dum = const.tile([128, 256], mm_dt, tag="dum")
        nc.any.memset(dum[:], 1.0)
        # 33rd GEMM2 k2-tile, per-chunk column slices: rows 0..7 get the
        # normalized gate each chunk, rows 8..127 stay zero forever
        hg32all = const.tile([128, BS], mm_dt, tag="hg32all")
        nc.any.memset(hg32all[:], 0.0)

        xp = ctx.enter_context(tc.tile_pool(name="xp", bufs=2))
        gm = ctx.enter_context(tc.tile_pool(name="gm", bufs=2))
        gbcp = ctx.enter_context(tc.tile_pool(name="gbcp", bufs=2))
        hgp = ctx.enter_context(tc.tile_pool(name="hgp", bufs=1))
        tmpp = ctx.enter_context(tc.tile_pool(name="tmpp", bufs=3))
        w2p = ctx.enter_context(tc.tile_pool(name="w2p", bufs=3))
        outp = ctx.enter_context(tc.tile_pool(name="outp", bufs=3))
        p1 = ctx.enter_context(tc.tile_pool(name="p1", bufs=4, space="PSUM"))
        p2 = ctx.enter_context(tc.tile_pool(name="p2", bufs=2, space="PSUM"))
        pbc = ctx.enter_context(tc.tile_pool(name="pbc", bufs=2, space="PSUM"))

        # warm-up: keep the PE busy from queue-up (~7us) until real
        # operands land so HAM reaches 8/8 before the first real matmul
        for i in range(NDUM):
            pd = p1.tile([128, CHUNK], f32, tag="ps1", name=f"dum_{i}")
            nc.tensor.matmul(pd[:, :256], dum[:, :128], dum[:, :256],
                             start=True, stop=True)

        # small consts first so the gate phase isn't stuck behind bulk DMA
        cst_sb = const.tile([128, 1 + HT], f32, tag="cst")
        nc.sync.dma_start(cst_sb[:], cst[:, :])
        wg_all = const.tile([128, KT * R], mm_dt, tag="wg_all")
        nc.sync.dma_start(wg_all[:], wgt[:, :])

        xtiles = {}
        xdmas = {}

        def emit_x_prefetch(c, split=False, dep=None):
            xt = xp.tile([128, KT * CHUNK], mm_dt, tag="xt", name=f"xt_{c}")
            half = KT * CHUNK // 2
            if split:
                d1 = nc.sync.dma_start(xt[:, :half], xT4[c, :, :half])
                d2 = nc.sync.dma_start(xt[:, half:], xT4[c, :, half:])
                xdmas[(c, 'a')] = d1
                last = d2
            else:
                last = nc.sync.dma_start(xt[:], xT4[c, :, :])
            if dep is not None:
                tile.add_dep_helper(last.ins, dep,
                                    reason=f"x({c}) after bulk w1")
            xtiles[c] = [xt[:, k * CHUNK:(k + 1) * CHUNK] for k in range(KT)]
            xdmas[c] = last

        gate_ns = {}
        gbcs = {}

        def emit_gate(c):
            """gate_n[c] = softmax(x@Wg.T+bg) rows, pre-normalized bf16."""
            sl = slice(c * CHUNK, (c + 1) * CHUNK)
            pg = pbc.tile([R, CHUNK], f32, tag="pb", name=f"pg_{c}")
            for k in range(KT):
                nc.tensor.matmul(pg[:], wg_all[:, k * R:(k + 1) * R],
                                 xtiles[c][k], start=(k == 0),
                                 stop=(k == KT - 1))
            E = gm.tile([R, CHUNK], mm_dt, tag="E", name=f"E_{c}")
            nc.scalar.activation(E[:], pg[:], AF.Exp, bias=cst_sb[0:R, 0:1])
            pS = pbc.tile([R, CHUNK], f32, tag="pb", name=f"pS_{c}")
            nc.tensor.matmul(pS[:], ones8x8[:], E[:], start=True, stop=True)
            rec = gm.tile([R, CHUNK], f32, tag="rec", name=f"rec_{c}")
            nc.vector.reciprocal(rec[:], pS[:])
            gn = gm.tile([R, CHUNK], mm_dt, tag="gn", name=f"gn_{c}")
            nc.vector.tensor_mul(gn[:], E[:], rec[:])
            gate_ns[c] = gn
            nc.vector.tensor_copy(hg32all[0:R, sl], gn[:])

        def emit_gate_bcast_pe(c):
            """chunk-0 path: replicate gate rows to 128 partitions via PE
            rank-1 matmuls (no DMA in the startup window); psums alternate
            between two pools so the MM/copy ping-pong stays 4 deep."""
            g = gbcp.tile([128, R * CHUNK], mm_dt, tag="gbca", name=f"gbc_{c}")
            for r in range(R):
                pool = pbc if r % 2 == 0 else p2
                pb = pool.tile([128, CHUNK], f32,
                               tag="pb" if r % 2 == 0 else "ps2",
                               name=f"pgb{c}_{r}")
                nc.tensor.matmul(pb[:], sel[:, r * 128:(r + 1) * 128],
                                 gate_ns[c][:], start=True, stop=True)
                # drain on two queues so the psum WAR never gates the PE
                if r % 2 == 0:
                    nc.vector.tensor_copy(g[:, r * CHUNK:(r + 1) * CHUNK],
                                          pb[:])
                else:
                    nc.scalar.activation(g[:, r * CHUNK:(r + 1) * CHUNK],
                                         pb[:], AF.Copy)
            gbcs[c] = g

        def emit_gate_bcast_dma(c):
            """steady-state path: one DRAM write + one replicating read on
            the (idle) GpSimd queue."""
            wr = nc.gpsimd.dma_start(gate_scr[c, :, :], gate_ns[c][:])
            g = gbcp.tile([128, R * CHUNK], mm_dt, tag="gbca", name=f"gbc_{c}")
            src = bass.AP(gate_scr, c * R * CHUNK, [[0, 128], [1, R * CHUNK]])
            rd = nc.gpsimd.dma_start(g[:], src)
            tile.add_dep_helper(rd.ins, wr.ins,
                                reason="gate bcast read after scr write")
            gbcs[c] = g

        # startup: x(0) split in halves for an early gate start, then the
        # w1 waves chained so they arrive in order
        emit_x_prefetch(0, split=True)
        nc.sync.dma_start(sel[:], seld[:, :])
        emit_gate(0)

        w1_all = const.tile([128, KT * RH], mm_dt, tag="w1all")
        wave_dma = []
        for q in range(NWAVE):
            lo = KT * WAVE_HT[q] * 128
            hi = KT * WAVE_HT[q + 1] * 128
            d = nc.sync.dma_start(w1_all[:, lo:hi], w1wd[:, lo:hi])
            # skip-one chaining: two waves in flight hides the issue/sem
            # handoff latency while preserving approximate arrival order
            dep = xdmas[(0, 'a')] if q < 2 else wave_dma[q - 2]
            tile.add_dep_helper(d.ins, dep.ins, reason=f"w1 wave {q} order")
            wave_dma.append(d)
        w1_last = wave_dma[-1]

        for c in range(NCHUNK):
            sl = slice(c * CHUNK, (c + 1) * CHUNK)
            xts = xtiles.pop(c)
            if c + 1 < NCHUNK:
                emit_x_prefetch(c + 1,
                                dep=w1_last.ins if c == 0 else None)

            def emit_g1_mms(ht):
                ps1 = p1.tile([128, CHUNK], f32, tag="ps1")
                for k in range(KT):
                    nc.tensor.matmul(ps1[:],
                                     w1_all[:, _w1_col(k, ht):
                                            _w1_col(k, ht) + 128],
                                     xts[k],
                                     start=(k == 0), stop=(k == KT - 1))
                return ps1

            def emit_g1_evict(ht, ps1, hgs):
                tmp = tmpp.tile([128, CHUNK], f32, tag="tmp",
                                name=f"tmp_{c}_{ht}")
                nc.scalar.activation(tmp[:], ps1[:], AF.Relu,
                                     bias=cst_sb[:, 1 + ht:2 + ht])
                hg = hgp.tile([128, CHUNK], mm_dt, tag=f"hg{ht}",
                              name=f"hg{ht}_{c}")
                r = ht // GRP
                nc.vector.tensor_mul(hg[:], tmp[:],
                                     gbcs[c][:, r * CHUNK:(r + 1) * CHUNK])
                hgs.append(hg[:])

            hgs = []
            ht_start = 0
            if c == 0:
                # keep the PE streaming while the gate's cross-engine
                # latency chain resolves: run the first h-tiles now, emit
                # the gate broadcast MMs, only then evict
                held = []
                for ht in range(HOLD0):
                    held.append(emit_g1_mms(ht))
                emit_gate_bcast_pe(0)
                for ht, ps1 in enumerate(held):
                    emit_g1_evict(ht, ps1, hgs)
                ht_start = HOLD0
            for ht in range(ht_start, HT):
                ps1 = emit_g1_mms(ht)
                emit_g1_evict(ht, ps1, hgs)
                # next chunk's gate chain, scattered mid-GEMM1 so the PE
                # stays fed while ACT/DVE/DMA latency hides
                if c + 1 < NCHUNK:
                    if ht == (16 if c == 0 else 10):
                        emit_gate(c + 1)
                    elif ht == 18:
                        emit_gate_bcast_dma(c + 1)
            hgs.append(hg32all[:, sl])

            for ot in range(OT):
                w2t = w2p.tile([128, RHE], mm_dt, tag="w2t")
                dma = nc.sync.dma_start(w2t[:], w2e[ot, :, :])
                if c == 0:
                    tile.add_dep_helper(dma.ins, w1_last.ins,
                                        reason="w2 stream after w1 bulk load")
                ps2 = p2.tile([128, CHUNK], f32, tag="ps2")
                for k2 in range(K2T + 1):
                    nc.tensor.matmul(ps2[:],
                                     w2t[:, k2 * 128:(k2 + 1) * 128],
                                     hgs[k2],
                                     start=(k2 == 0), stop=(k2 == K2T))
                osb = outp.tile([128, CHUNK], f32, tag="osb")
                nc.vector.tensor_copy(osb[:], ps2[:])
                nc.scalar.dma_start(outT[ot, :, sl], osb[:])
            del gbcs[c], gate_ns[c]

    nc.compile()
    return nc


def _get_nc(mm_dt_name="bfloat16"):
    if mm_dt_name not in _NC_CACHE:
        _NC_CACHE[mm_dt_name] = _build_nc(mm_dt_name)
    return _NC_CACHE[mm_dt_name]


def _prepare_in_maps(inputs, np_mm_dtype):
    x = np.asarray(inputs["x"], np.float32)
    in_proj_w = np.asarray(inputs["in_proj_w"], np.float32)
    in_proj_b = np.asarray(inputs["in_proj_b"], np.float32)
    out_proj_w = np.asarray(inputs["out_proj_w"], np.float32)
    out_proj_b = np.asarray(inputs["out_proj_b"], np.float32)
    W1 = np.asarray(inputs["W1"], np.float32)
    b1 = np.asarray(inputs["b1"], np.float32)
    W2 = np.asarray(inputs["W2"], np.float32)
    b2 = np.asarray(inputs["b2"], np.float32)
    Wg = np.asarray(inputs["Wg"], np.float32)
    bg = np.asarray(inputs["bg"], np.float32)

    Wv = in_proj_w[2 * D:]
    bv = in_proj_b[2 * D:]
    A = out_proj_w @ Wv                       # [D, D]
    ba = out_proj_w @ bv + out_proj_b         # [D]
    W1r = W1.reshape(RH, D)
    W1f = W1r @ A                             # [RH, D]
    b1f = W1r @ ba + b1.reshape(RH)           # [RH]
    W2cat = W2.transpose(0, 2, 1).reshape(RH, DOUT)

    # w1 in wave-major blocks: block q is [128, KT*width_q], cols k-major
    w1t = np.ascontiguousarray(W1f.T).reshape(KT, 128, RH)
    blocks = []
    for q in range(NWAVE):
        lo, hi = WAVE_HT[q] * 128, WAVE_HT[q + 1] * 128
        blocks.append(w1t[:, :, lo:hi].transpose(1, 0, 2)
                      .reshape(128, KT * (hi - lo)))
    w1wd_np = np.ascontiguousarray(np.concatenate(blocks, axis=1))

    # w2 with b2 appended as a 33rd K-tile (rows 0..7 = b2, rest zero)
    w2base = np.ascontiguousarray(
        W2cat.reshape(K2T, 128, OT, 128).transpose(2, 1, 0, 3)
    ).reshape(OT, 128, RH)
    w2e_np = np.zeros((OT, 128, RHE), np.float32)
    w2e_np[:, :, :RH] = w2base
    # stationary block for k2=32: [128, 128] with rows 0..7 = b2[:, ocols]
    b2blk = b2.reshape(R, OT, 128).transpose(1, 0, 2)   # [OT, R, 128]
    w2e_np[:, 0:R, RH:] = b2blk

    # [p, k*R+r] = Wg[r, k*128+p]: contiguous per partition line
    wgt_np = np.ascontiguousarray(Wg.reshape(R, KT, 128).transpose(2, 1, 0)
                                  ).reshape(128, KT * R)
    # broadcast selectors: row r of block r is all-ones
    seld_np = np.zeros((R, R * 128), np.float32)
    for r in range(R):
        seld_np[r, r * 128:(r + 1) * 128] = 1.0
    # packed f32 consts: col 0 = bg (rows 0..7), cols 1.. = b1f tiles
    cst_np = np.zeros((128, 1 + HT), np.float32)
    cst_np[0:R, 0] = bg
    cst_np[:, 1:] = b1f.reshape(HT, 128).T

    shared = {
        "w1wd": w1wd_np.astype(np_mm_dtype),
        "w2e": w2e_np.astype(np_mm_dtype),
        "wgt": wgt_np.astype(np_mm_dtype),
        "seld": seld_np.astype(np_mm_dtype),
        "cst": cst_np,
    }
    in_maps = []
    for cr in range(NCORES):
        xs = x[cr * BS:(cr + 1) * BS]         # [BS, D]
        xT_np = np.empty((NCHUNK, 128, KT * CHUNK), np.float32)
        for c in range(NCHUNK):
            xc = xs[c * CHUNK:(c + 1) * CHUNK]            # [CHUNK, D]
            xT_np[c] = (xc.T.reshape(KT, 128, CHUNK).transpose(1, 0, 2)
                        .reshape(128, KT * CHUNK))
        m = dict(shared)
        m["xT4"] = xT_np.astype(np_mm_dtype)
        in_maps.append(m)
    return in_maps


def _run(inputs, trace=False, mm_dt_name="bfloat16"):
    import ml_dtypes
    from concourse.bass_utils import run_bass_kernel_spmd

    np_mm = ml_dtypes.bfloat16 if mm_dt_name == "bfloat16" else np.float32
    nc = _get_nc(mm_dt_name)
    in_maps = _prepare_in_maps(inputs, np_mm)
    res = run_bass_kernel_spmd(nc, in_maps, list(range(NCORES)), trace=trace)
    out = np.empty((B, DOUT), np.float32)
    for c in range(NCORES):
        out[c * BS:(c + 1) * BS] = res.results[c]["outT"].reshape(DOUT, BS).T
    return out, res


def kernel(**inputs):
    out, _ = _run(inputs, trace=False)
    return out
